# revision 5
# baseline (speedup 1.0000x reference)
"""Trainium2 Bass kernel for nn_LSH: ret[o] = sum_{s,a} x[s] * w[o,s,a].

x: [1, 4096] f32, weights: [512, 4096, 128] f32 -> ret: [512] f32.

Sharding: out_dim 512 is split 64-per-core across 8 cores; x is replicated.

Per core the weights slice is uploaded pre-transposed/interleaved in fp8
e4m3, quartering the HBM stream to 32 MiB; that stream is the roofline
(the 16 SDMA engines run back-to-back at line rate for ~87 us).

The contraction runs on the tensor engine in perf_mode=DoubleRow (fp8
stationary x fp8 moving, 2 MACs/cell/cycle): each matmul contracts 256
s-values (128 partitions x 2 planes) for 512 (o, a) columns, so the whole
stream needs 256 matmuls instead of 512 and the tensor engine stays ahead
of the DMA stream instead of draining ~17 us past it (the baseline bf16
stationary ran the PE at 1 col/cycle). DoubleRow is incompatible with PE
column tiling (walrus emits full-array col_grp, so the psum destination
must start at partition 0): all matmuls use one stationary footprint at
tile_position (0, 0) writing psum partitions 0-31.

Numerics: the stationary is xq = e4m3(x/2) (quantized x). Its rounding
error is compensated exactly on the host by pre-scaling each weight row s
by 8*x[s]/xq[s] (folding the previous x16 fp8 weight scale) before the
error-diffused fp8 weight quantization, so the device computes
sum_s xq[s]*wq[o,s,a] ~= 8*ret[o] with only the weight-diffusion residual
left (the 1/8 is folded into the final selector matmul). The diffusion
runs along the innermost a axis as before: each element stays within one
quantization step of its target and the per-(o,s) residual telescopes.
Measured end-to-end max-rel error on the seeded inputs is 3.0e-3 against
the 2e-2 gate.

Layout: s is split into 16 chunks of 256; chunk c maps s = 256c+128i+k to
partition k, DoubleRow plane i; stationary column m = (128i+k)//8 groups 8
s-values. Per o-half (32 outputs) a quad DMA carries 2 chunks as [128
partitions x 16 KiB contiguous] (2 MiB), free layout [c(2), j(8), i(2),
olo(4), a(128)] so each matmul's moving operand is one contiguous 1
KiB-per-partition slab viewed as [128, 2, 512]. Matmul j accumulates all
16 chunks of its half into psum bank j (cols 512j..512j+512); the two
halves reuse the banks back-to-back: per-bank DVE reduces over a
([32, 4, 128] -> [32, 4] into red[:, 32h+4j..]) start as soon as bank j's
last matmul retires, and half B's bank-j start matmul WAR-depends only on
half A's bank-j reduce, so the handoff costs ~0. A final fp32 matmul
against a 0.125-scaled ones vector folds the 32 group-partitions into
out[64, 1] = ret (identity order), written via a psum slice reuse.
"""

import sys

sys.path.insert(0, "/opt/trn_rl_repo")

import ml_dtypes
import numpy as np

import concourse.bass as bass
import concourse.mybir as mybir
import concourse.tile as tile
from concourse import bacc
from concourse.bass_utils import run_bass_kernel_spmd

FP8 = ml_dtypes.float8_e4m3

P = 128
O_PER_CORE = 64
O_HALF = 32
N_CORES = 8
S = 4096
A = 128
NCH = 16  # s-chunks of 256 (each spans the full s range once per half)
CHS = 256  # s per chunk (128 partitions x 2 DoubleRow planes)
M = 32  # stationary columns / psum partitions
HCOLS = O_HALF * A  # 4096 (o, a) columns per chunk and o-half
NMM = HCOLS // 512  # 8 matmuls of N=512 out-cols per (chunk, half)
NQ = 8  # quad DMAs (two chunks of one half) per half
QBYTES = 2 * 2 * HCOLS  # 16384 fp8 per partition per quad

_CACHED_NC = None


def _build_nc():
    nc = bacc.Bacc(
        "TRN2",
        target_bir_lowering=False,
        debug=False,
        num_devices=N_CORES,
    )
    w8 = nc.dram_tensor(
        "w8", [2 * NQ * P, QBYTES], mybir.dt.float8e4, kind="ExternalInput"
    ).ap()
    xg = nc.dram_tensor(
        "xg", [P, NCH * 2 * M], mybir.dt.float8e4, kind="ExternalInput"
    ).ap()
    sel = nc.dram_tensor("sel", [M, 1], mybir.dt.float32, kind="ExternalInput").ap()
    out = nc.dram_tensor("out", [O_PER_CORE, 1], mybir.dt.float32, kind="ExternalOutput").ap()

    with tile.TileContext(nc) as tc:
        with (
            tc.tile_pool(name="wp8", bufs=11) as wp8,
            tc.tile_pool(name="const", bufs=1) as constp,
            tc.tile_pool(name="accp", bufs=1) as accp,
            tc.tile_pool(name="psum", bufs=1, space="PSUM") as psp,
        ):
            xg_t = constp.tile([P, NCH * 2 * M], mybir.dt.float8e4)
            sel_t = constp.tile([M, 1], mybir.dt.float32)
            ps = psp.tile([P, 8 * 512], mybir.dt.float32)
            red = accp.tile([M, 2 * O_HALF], mybir.dt.float32)
            res = accp.tile([O_PER_CORE, 1], mybir.dt.float32)

            # All DMAs ride the two HWDGE rings: any SWDGE (gpsimd)
            # activity slows SDMA engines 7/15 via descriptor-ring AXI
            # port contention, and the slowest engine paces the whole
            # now-DMA-bound stream. xg is tiny and lands first.
            first_wt = wp8.tile([P, QBYTES], mybir.dt.float8e4, tag="wt8")
            nc.sync.dma_start(xg_t[:], xg[:])
            nc.scalar.dma_start(sel_t[:], sel[:])
            nc.sync.dma_start(first_wt[:], w8[0:P, :])

            i = 1
            for half in range(2):
                for ql in range(NQ):
                    if half == 0 and ql == 0:
                        wt = first_wt
                    else:
                        wt = wp8.tile([P, QBYTES], mybir.dt.float8e4, tag="wt8")
                        r0 = (half * NQ + ql) * P
                        # Alternate between the two physical HWDGE rings
                        # (SP and ACT) so the weight stream keeps both
                        # descriptor queues fed.
                        dma_eng = nc.sync if i % 2 == 0 else nc.scalar
                        i += 1
                        if ql < NQ - 1:
                            dma_eng.dma_start(wt[:], w8[r0 : r0 + P, :])
                        else:
                            # Split the last quad's DMA at the chunk
                            # boundary so chunk 14's matmuls start while
                            # chunk 15's bytes are still in flight.
                            hc = QBYTES // 2
                            dma_eng.dma_start(wt[:, :hc], w8[r0 : r0 + P, :hc])
                            dma_eng.dma_start(wt[:, hc:], w8[r0 : r0 + P, hc:])
                    for cl in range(2):
                        cg = 2 * ql + cl  # s-chunk within this half
                        lhs = xg_t[:, cg * 64 : (cg + 1) * 64].rearrange(
                            "p (i m) -> p i m", i=2
                        )
                        for j in range(NMM):
                            rhs = wt[
                                :, (cl * NMM + j) * 1024 : (cl * NMM + j + 1) * 1024
                            ].rearrange("p (i n) -> p i n", i=2)
                            nc.tensor.matmul(
                                ps[0:M, j * 512 : (j + 1) * 512],
                                lhs,
                                rhs,
                                start=(cg == 0),
                                stop=(cg == NCH - 1),
                                perf_mode=mybir.MatmulPerfMode.DoubleRow,
                                tile_position=(0, 0),
                                # The two halves reuse the banks; the
                                # sim's zero-region group check is
                                # coarser than the HW per-element
                                # has_written.
                                skip_group_check=True,
                            )
                            if cg == NCH - 1:
                                # Bank j is final for this half: fold a
                                # out, [32, 4, 128] -> [32, 4]. Half B's
                                # bank-j start matmul WAR-depends on this.
                                nc.vector.tensor_reduce(
                                    red[
                                        :,
                                        half * O_HALF + 4 * j : half * O_HALF
                                        + 4 * (j + 1),
                                    ],
                                    ps[0:M, j * 512 : (j + 1) * 512].rearrange(
                                        "p (o a) -> p o a", a=A
                                    ),
                                    axis=mybir.AxisListType.X,
                                    op=mybir.AluOpType.add,
                                )

            # Fold the 32 group-partitions via the 0.125-scaled ones
            # vector: out[o, 0] = sum_m red[m, o] / 8 (identity o order).
            # Reuses a psum slice (WAR on the reduces) - psum is full.
            nc.tensor.matmul(
                ps[0:O_PER_CORE, 0:1], red[:], sel_t[:], start=True, stop=True
            )
            nc.scalar.copy(res[:], ps[0:O_PER_CORE, 0:1])
            # HWDGE (sync ring, idle by now) beats SWDGE's ~1.6us Q7
            # emission latency for the final 256 B store.
            nc.sync.dma_start(out[:], res[:])

    nc.compile()
    return nc


def _get_nc():
    global _CACHED_NC
    if _CACHED_NC is None:
        _CACHED_NC = _build_nc()
    return _CACHED_NC


def _fp8_diffuse(block):
    """Quantize [..., A] targets to fp8 codes with 1-D error feedback
    along the last axis (dithered rounding; every element stays within
    one quantization step of its target)."""
    src = np.asarray(block, dtype=np.float64)
    codes = np.empty(block.shape, dtype=FP8)
    carry = np.zeros(block.shape[:-1])
    for a in range(block.shape[-1]):
        t = src[..., a] + carry
        qa = t.astype(np.float32).astype(FP8)
        carry = t - qa.astype(np.float64)
        codes[..., a] = qa
    return codes


def _in_maps(x, weights):
    x = np.ascontiguousarray(np.asarray(x, dtype=np.float32)).reshape(S)
    weights = np.asarray(weights, dtype=np.float32)

    # Stationary: xq = e4m3(x/2); its rounding error is compensated in
    # the weight targets below via ratio = 8*x/xq (so xq*wq ~= 8*x*w).
    xq = (x.astype(np.float64) / 2).astype(np.float32).astype(FP8)
    xqf = xq.astype(np.float64)
    safe = np.where(xqf != 0.0, xqf, 1.0)
    ratio = np.where(xqf != 0.0, 8.0 * x.astype(np.float64) / safe, 16.0)

    # xg[k, c, i, m] = xq[256c + 128i + k] at column m = (128i + k)//8.
    xg = np.zeros((P, NCH, 2, M), dtype=FP8)
    r = np.arange(CHS)
    for c in range(NCH):
        xg[r % P, c, r // P, r // 8] = xq[c * CHS + r]
    xg = np.ascontiguousarray(xg).reshape(P, NCH * 2 * M)

    sel = np.full((M, 1), 0.125, dtype=np.float32)  # folds the 1/8 scale

    maps = []
    for core in range(N_CORES):
        wc = weights[core * O_PER_CORE : (core + 1) * O_PER_CORE]
        tr = wc.transpose(1, 0, 2)  # [s, o, a] fp32 view
        tgt = tr.astype(np.float64) * ratio[:, None, None]
        codes = _fp8_diffuse(tgt)  # [s, o, a] fp8 codes

        # [ql, c, i, k, half, j, olo, a] -> [half, ql, k, c, j, i, olo, a]
        # so each quad DMA is [128 partitions x 16 KiB contiguous] and
        # each matmul's moving slab is 1 KiB/partition contiguous.
        sview = codes.reshape(NQ, 2, 2, P, 2, NMM, 4, A)
        wcore = sview.transpose(4, 0, 3, 1, 5, 2, 6, 7)
        maps.append(
            {
                "w8": np.ascontiguousarray(wcore).reshape(2 * NQ * P, QBYTES),
                "xg": xg,
                "sel": sel,
            }
        )
    return maps


def run(x, weights, trace=False):
    """Run on hardware; returns (ret[512], BassKernelResults)."""
    nc = _get_nc()
    res = run_bass_kernel_spmd(
        nc, _in_maps(x, weights), list(range(N_CORES)), trace=trace
    )
    ret = np.concatenate(
        [res.results[c]["out"].reshape(O_PER_CORE) for c in range(N_CORES)]
    ).astype(np.float32)
    return ret, res


def kernel(x, weights):
    ret, _ = run(x, weights)
    return ret


# revision 8
# speedup vs baseline: 1.1592x; 1.1592x over previous
"""Trainium2 Bass kernel for nn_LSH: ret[o] = sum_{s,a} x[s] * w[o,s,a].

x: [1, 4096] f32, weights: [512, 4096, 128] f32 -> ret: [512] f32.

Sharding: out_dim 512 is split 64-per-core across 8 cores; x is replicated.

Per core the weights slice is uploaded pre-transposed/interleaved in fp8
e4m3, quartering the HBM stream to 32 MiB; that stream is the roofline
(the 16 SDMA engines run back-to-back at line rate for ~87 us).

The contraction runs on the tensor engine in perf_mode=DoubleRow (fp8
stationary x fp8 moving, 2 MACs/cell/cycle): each matmul contracts 256
s-values (128 partitions x 2 planes) for 512 (o, a) columns, so the whole
stream needs 256 matmuls instead of 512 and the tensor engine stays ahead
of the DMA stream instead of draining ~17 us past it (the baseline bf16
stationary ran the PE at 1 col/cycle). DoubleRow is incompatible with PE
column tiling (walrus emits full-array col_grp, so the psum destination
must start at partition 0): all matmuls use one stationary footprint at
tile_position (0, 0) writing psum partitions 0-31.

Numerics: the stationary is xq = e4m3(x/2) (quantized x). Its rounding
error is compensated exactly on the host by pre-scaling each weight row s
by 8*x[s]/xq[s] (folding the previous x16 fp8 weight scale) before the
error-diffused fp8 weight quantization, so the device computes
sum_s xq[s]*wq[o,s,a] ~= 8*ret[o] with only the weight-diffusion residual
left (the 1/8 is folded into the final selector matmul). The diffusion
runs along the innermost a axis as before: each element stays within one
quantization step of its target and the per-(o,s) residual telescopes.
Measured end-to-end max-rel error on the seeded inputs is 3.0e-3 against
the 2e-2 gate.

Layout: s is split into 16 chunks of 256; chunk c maps s = 256c+128i+k to
partition k, DoubleRow plane i; stationary column m = (128i+k)//8 groups 8
s-values. Per o-half (32 outputs) a quad DMA carries 2 chunks as [128
partitions x 16 KiB contiguous] (2 MiB), free layout [c(2), j(8), i(2),
olo(4), a(128)] so each matmul's moving operand is one contiguous 1
KiB-per-partition slab viewed as [128, 2, 512]. Matmul j accumulates all
16 chunks of its half into psum bank j (cols 512j..512j+512); the two
halves reuse the banks back-to-back: per-bank DVE reduces over a
([32, 4, 128] -> [32, 4] into red[:, 32h+4j..]) start as soon as bank j's
last matmul retires, and half B's bank-j start matmul WAR-depends only on
half A's bank-j reduce, so the handoff costs ~0. A final fp32 matmul
against a 0.125-scaled ones vector folds the 32 group-partitions into
out[64, 1] = ret (identity order), written via a psum slice reuse.
"""

import sys

sys.path.insert(0, "/opt/trn_rl_repo")

import ml_dtypes
import numpy as np

import concourse.bass as bass
import concourse.mybir as mybir
import concourse.tile as tile
from concourse import bacc
from concourse.bass_utils import run_bass_kernel_spmd

FP8 = ml_dtypes.float8_e4m3

P = 128
O_PER_CORE = 64
O_HALF = 32
N_CORES = 8
S = 4096
A = 128
NCH = 16  # s-chunks of 256 (each spans the full s range once per half)
CHS = 256  # s per chunk (128 partitions x 2 DoubleRow planes)
M = 32  # stationary columns / psum partitions
HCOLS = O_HALF * A  # 4096 (o, a) columns per chunk and o-half
NMM = HCOLS // 512  # 8 matmuls of N=512 out-cols per (chunk, half)
NQ = 8  # quad DMAs (two chunks of one half) per half
QBYTES = 2 * 2 * HCOLS  # 16384 fp8 per partition per quad

_CACHED_NC = None


def _build_nc():
    nc = bacc.Bacc(
        "TRN2",
        target_bir_lowering=False,
        debug=False,
        num_devices=N_CORES,
    )
    w8 = nc.dram_tensor(
        "w8", [2 * NQ * P, QBYTES], mybir.dt.float8e4, kind="ExternalInput"
    ).ap()
    xg = nc.dram_tensor(
        "xg", [P, NCH * 2 * M], mybir.dt.float8e4, kind="ExternalInput"
    ).ap()
    sel = nc.dram_tensor("sel", [M, 1], mybir.dt.float32, kind="ExternalInput").ap()
    out = nc.dram_tensor("out", [O_PER_CORE, 1], mybir.dt.float32, kind="ExternalOutput").ap()

    with tile.TileContext(nc) as tc:
        with (
            tc.tile_pool(name="wp8", bufs=11) as wp8,
            tc.tile_pool(name="const", bufs=1) as constp,
            tc.tile_pool(name="accp", bufs=1) as accp,
            tc.tile_pool(name="psum", bufs=1, space="PSUM") as psp,
        ):
            xg_t = constp.tile([P, NCH * 2 * M], mybir.dt.float8e4)
            sel_t = constp.tile([M, 1], mybir.dt.float32)
            ps = psp.tile([P, 8 * 512], mybir.dt.float32)
            red = accp.tile([M, 2 * O_HALF], mybir.dt.float32)
            res = accp.tile([O_PER_CORE, 1], mybir.dt.float32)
            rscr = accp.tile([M, A], mybir.dt.float32)  # activation scratch

            # All DMAs ride the two HWDGE rings: any SWDGE (gpsimd)
            # activity slows SDMA engines 7/15 via descriptor-ring AXI
            # port contention, and the slowest engine paces the whole
            # now-DMA-bound stream. xg is tiny and lands first.
            first_wt = wp8.tile([P, QBYTES], mybir.dt.float8e4, tag="wt8")
            nc.sync.dma_start(xg_t[:], xg[:])
            nc.scalar.dma_start(sel_t[:], sel[:])
            nc.sync.dma_start(first_wt[:], w8[0:P, :])

            i = 1
            for half in range(2):
                for ql in range(NQ):
                    if half == 0 and ql == 0:
                        wt = first_wt
                    else:
                        wt = wp8.tile([P, QBYTES], mybir.dt.float8e4, tag="wt8")
                        r0 = (half * NQ + ql) * P
                        # Alternate between the two physical HWDGE rings
                        # (SP and ACT) so the weight stream keeps both
                        # descriptor queues fed. The last two quads ride
                        # the same (scalar) ring so their per-engine FIFO
                        # delivers them in order and the tensor's final
                        # drain is one chunk deep, not three.
                        if half == 1 and ql >= NQ - 2:
                            dma_eng = nc.scalar
                        else:
                            dma_eng = nc.sync if i % 2 == 0 else nc.scalar
                        i += 1
                        if half == 1 and ql == NQ - 1:
                            # Split the last quad's DMA at the chunk
                            # boundary so chunk 14's matmuls start while
                            # chunk 15's bytes are still in flight.
                            hc = QBYTES // 2
                            dma_eng.dma_start(wt[:, :hc], w8[r0 : r0 + P, :hc])
                            dma_eng.dma_start(wt[:, hc:], w8[r0 : r0 + P, hc:])
                        else:
                            dma_eng.dma_start(wt[:], w8[r0 : r0 + P, :])
                    for cl in range(2):
                        cg = 2 * ql + cl  # s-chunk within this half
                        lhs = xg_t[:, cg * 64 : (cg + 1) * 64].rearrange(
                            "p (i m) -> p i m", i=2
                        )
                        for j in range(NMM):
                            # Moving slab holds the DoubleRow plane pairs
                            # adjacent ([n, i] innermost) so the PE fetch
                            # walks SBUF strictly sequentially.
                            rhs = wt[
                                :, (cl * NMM + j) * 1024 : (cl * NMM + j + 1) * 1024
                            ].rearrange("p (n i) -> p i n", i=2)
                            nc.tensor.matmul(
                                ps[0:M, j * 512 : (j + 1) * 512],
                                lhs,
                                rhs,
                                start=(cg == 0),
                                stop=(cg == NCH - 1),
                                perf_mode=mybir.MatmulPerfMode.DoubleRow,
                                tile_position=(0, 0),
                                # The two halves reuse the banks; the
                                # sim's zero-region group check is
                                # coarser than the HW per-element
                                # has_written.
                                skip_group_check=True,
                            )
                            if cg == NCH - 1:
                                # Bank j is final for this half: fold a
                                # out, [32, 4, 128] -> [32, 4]. Half B's
                                # bank-j start matmul WAR-depends on this.
                                # Half A stays all-vector (scalar-issued
                                # reduces would block that sequencer's
                                # remaining quad issues); half B's odd
                                # banks fold on the scalar engine via
                                # activation row-accumulate so the two
                                # engines drain the tail in parallel.
                                rslice = red[
                                    :,
                                    half * O_HALF + 4 * j : half * O_HALF
                                    + 4 * (j + 1),
                                ]
                                if half == 1 and j % 2 == 1:
                                    for olo in range(4):
                                        nc.scalar.activation(
                                            rscr[:],
                                            ps[
                                                0:M,
                                                j * 512 + olo * A : j * 512
                                                + (olo + 1) * A,
                                            ],
                                            mybir.ActivationFunctionType.Copy,
                                            accum_out=rslice[:, olo : olo + 1],
                                        )
                                else:
                                    nc.vector.tensor_reduce(
                                        rslice,
                                        ps[0:M, j * 512 : (j + 1) * 512].rearrange(
                                            "p (o a) -> p o a", a=A
                                        ),
                                        axis=mybir.AxisListType.X,
                                        op=mybir.AluOpType.add,
                                    )

            # Fold the 32 group-partitions via the 0.125-scaled ones
            # vector: out[o, 0] = sum_m red[m, o] / 8 (identity o order).
            # Reuses a psum slice (WAR on the reduces) - psum is full.
            nc.tensor.matmul(
                ps[0:O_PER_CORE, 0:1], red[:], sel_t[:], start=True, stop=True
            )
            nc.scalar.copy(res[:], ps[0:O_PER_CORE, 0:1])
            # HWDGE (sync ring, idle by now) beats SWDGE's ~1.6us Q7
            # emission latency for the final 256 B store.
            nc.sync.dma_start(out[:], res[:])

    nc.compile()
    return nc


def _get_nc():
    global _CACHED_NC
    if _CACHED_NC is None:
        _CACHED_NC = _build_nc()
    return _CACHED_NC


def _fp8_diffuse(block):
    """Quantize [..., A] targets to fp8 codes with 1-D error feedback
    along the last axis (dithered rounding; every element stays within
    one quantization step of its target)."""
    src = np.asarray(block, dtype=np.float64)
    codes = np.empty(block.shape, dtype=FP8)
    carry = np.zeros(block.shape[:-1])
    for a in range(block.shape[-1]):
        t = src[..., a] + carry
        qa = t.astype(np.float32).astype(FP8)
        carry = t - qa.astype(np.float64)
        codes[..., a] = qa
    return codes


def _in_maps(x, weights):
    x = np.ascontiguousarray(np.asarray(x, dtype=np.float32)).reshape(S)
    weights = np.asarray(weights, dtype=np.float32)

    # Stationary: xq = e4m3(x/2); its rounding error is compensated in
    # the weight targets below via ratio = 8*x/xq (so xq*wq ~= 8*x*w).
    xq = (x.astype(np.float64) / 2).astype(np.float32).astype(FP8)
    xqf = xq.astype(np.float64)
    safe = np.where(xqf != 0.0, xqf, 1.0)
    ratio = np.where(xqf != 0.0, 8.0 * x.astype(np.float64) / safe, 16.0)

    # xg[k, c, i, m] = xq[256c + 128i + k] at column m = (128i + k)//8.
    xg = np.zeros((P, NCH, 2, M), dtype=FP8)
    r = np.arange(CHS)
    for c in range(NCH):
        xg[r % P, c, r // P, r // 8] = xq[c * CHS + r]
    xg = np.ascontiguousarray(xg).reshape(P, NCH * 2 * M)

    sel = np.full((M, 1), 0.125, dtype=np.float32)  # folds the 1/8 scale

    maps = []
    for core in range(N_CORES):
        wc = weights[core * O_PER_CORE : (core + 1) * O_PER_CORE]
        tr = wc.transpose(1, 0, 2)  # [s, o, a] fp32 view
        tgt = tr.astype(np.float64) * ratio[:, None, None]
        codes = _fp8_diffuse(tgt)  # [s, o, a] fp8 codes

        # [ql, c, i, k, half, j, olo, a] -> [half, ql, k, c, j, olo, a, i]
        # so each quad DMA is [128 partitions x 16 KiB contiguous] and
        # each matmul's moving slab is 1 KiB/partition contiguous with
        # the DoubleRow plane pairs adjacent.
        sview = codes.reshape(NQ, 2, 2, P, 2, NMM, 4, A)
        wcore = sview.transpose(4, 0, 3, 1, 5, 6, 7, 2)
        maps.append(
            {
                "w8": np.ascontiguousarray(wcore).reshape(2 * NQ * P, QBYTES),
                "xg": xg,
                "sel": sel,
            }
        )
    return maps


def run(x, weights, trace=False):
    """Run on hardware; returns (ret[512], BassKernelResults)."""
    nc = _get_nc()
    res = run_bass_kernel_spmd(
        nc, _in_maps(x, weights), list(range(N_CORES)), trace=trace
    )
    ret = np.concatenate(
        [res.results[c]["out"].reshape(O_PER_CORE) for c in range(N_CORES)]
    ).astype(np.float32)
    return ret, res


def kernel(x, weights):
    ret, _ = run(x, weights)
    return ret
